# revision 1
# baseline (speedup 1.0000x reference)
"""Trainium2 Bass kernel for the Mamba BasicBlock (nn_BasicBlock_59622736003616).

Full inputs in, full outputs out. Sharding: pure data-parallel over batch
(b=8 -> one batch element per NeuronCore, no collectives).

Per-core algorithm (l=2048 tokens, d_model=1024, d_inner=2048, n=16):
  LN -> transpose -> in_proj (bf16 PE) -> causal conv (DVE FMA taps) -> silu
     -> x_proj / dt_proj (PE) -> softplus
     -> selective scan: 16 native DVE tensor_tensor_scan ops per d-block
        (dA_n = exp(-n*dt) straight from ACT with scale=-n, since A[d,n]=-n)
     -> C-contraction + D-skip + z-gate -> out_proj (bf16 PE) -> +residual.

All ACT functions are drawn from the single `natural_log_exp_and_others`
table (exp/ln/square/copy) to avoid table-switch stalls:
  softplus(x) = ln(1+exp(x));  silu(x) = x*exp(-ln(1+exp(-x)));
  rsqrt(v) = exp(-0.5*ln(v)).
Time is chunked (TC columns) so phase A (matmuls) of chunk k+1 overlaps the
scan phase of chunk k; dt/xm/silu(z) round-trip through DRAM scratch to keep
SBUF under budget.
"""

import numpy as np
import ml_dtypes

import concourse.bass as bass
import concourse.mybir as mybir
import concourse.tile as tile
from concourse import bacc
from concourse.bass_utils import run_bass_kernel_spmd

F32 = mybir.dt.float32
BF16 = mybir.dt.bfloat16
AF = mybir.ActivationFunctionType
ALU = mybir.AluOpType

D_MODEL = 1024
D_INNER = 2048
D_STATE = 16
D_CONV = 4
DT_RANK = 64
NB = 8           # batch == number of cores
L = 2048         # sequence length
TC = 512         # time-chunk columns
NDB = D_INNER // 128   # 16 d-blocks
NEG = (2 * D_INNER) // 128 // 8   # e-groups of 8x128 for in_proj (=4)
NKD = D_MODEL // 128   # 8 contraction blocks for in_proj
EPS = 1e-5


def _silu3(nc, accp, src_ap, tc_cols):
    """sigmoid(src) via 3 ACT ops in the ln/exp table; returns sig tile."""
    e1 = accp.tile([128, tc_cols], F32, tag="sil1", name="sil1")
    nc.scalar.activation(e1[:], src_ap, AF.Exp, scale=-1.0)
    nc.scalar.activation(e1[:], e1[:], AF.Ln, bias=1.0)
    nc.scalar.activation(e1[:], e1[:], AF.Exp, scale=-1.0)
    return e1


def _emit(tc_ctx, nc, ios, a_scales, ctx, ln_simple=False):
    """Emit the whole per-core program under TileContext tc_ctx."""
    p = lambda **kw: ctx.enter_context(tc_ctx.tile_pool(**kw))
    psum = bass.MemorySpace.PSUM

    # ---------------- pools ----------------
    const = p(name="const", bufs=1)
    xpool = p(name="x", bufs=2)
    lnsml = p(name="lnsml", bufs=8)
    cpool = p(name="c", bufs=2)
    hpool = p(name="h", bufs=2)
    tpsum = p(name="mmA", bufs=2, space=psum)      # transpose + dt_proj psum
    htp = p(name="ht", bufs=9)
    winp = p(name="win", bufs=9)
    xzp = p(name="xz", bufs=2, space=psum)
    rawp = p(name="raw", bufs=2)
    accp = p(name="cacc", bufs=2)
    xmp = p(name="xm", bufs=3)
    zp = p(name="z", bufs=3)
    xpp = p(name="xp", bufs=1, space=psum)
    btct = p(name="btct", bufs=2)
    dtrp = p(name="dtr", bufs=2)
    dttp = p(name="dtt", bufs=2)
    brep = p(name="brep", bufs=16)
    crep = p(name="crep", bufs=1)
    ldp = p(name="ld", bufs=3)
    up = p(name="u", bufs=2)
    dap = p(name="da", bufs=3)
    dbxp = p(name="dbx", bufs=3)
    hdp = p(name="hd", bufs=1)
    yaccp = p(name="yacc", bufs=1)
    ygsp = p(name="ygs", bufs=1)
    ygp = p(name="yg", bufs=17)
    woutp = p(name="wout", bufs=4)
    outps = p(name="outps", bufs=4, space=psum)
    outsb = p(name="outsb", bufs=2)
    xresp = p(name="xres", bufs=3)
    statep = p(name="state", bufs=1)

    # ---------------- constants / small weights resident ----------------
    ident = const.tile([128, 128], BF16)
    nc.sync.dma_start(ident[:], ios["ident"][:])
    lnw = const.tile([128, D_MODEL], BF16)
    nc.sync.dma_start(lnw[:], ios["lnw_rep"][:])
    lnb = const.tile([128, D_MODEL], BF16)
    nc.sync.dma_start(lnb[:], ios["lnb_rep"][:])
    conv4 = const.tile([128, NDB * D_CONV], F32)
    nc.sync.dma_start(conv4[:], ios["conv4"][:])
    convb = const.tile([128, NDB], F32)
    nc.sync.dma_start(convb[:], ios["convb"][:])
    dtb = const.tile([128, NDB], F32)
    nc.sync.dma_start(dtb[:], ios["dtb"][:])
    dmat = const.tile([128, NDB], F32)
    nc.sync.dma_start(dmat[:], ios["dmat"][:])
    wx = const.tile([128, NDB * 96], BF16)
    nc.sync.dma_start(wx[:], ios["wxT"][:])
    wdt = const.tile([64, D_INNER], BF16)
    nc.sync.dma_start(wdt[:], ios["wdtT"][:])
    epsb = const.tile([128, 1], F32)
    nc.vector.memset(epsb[:], EPS)

    # scan carry state (one column per (dblk, n)) and conv halo tails
    S = statep.tile([128, NDB * D_STATE], F32)
    nc.vector.memset(S[:], 0.0)
    tails = statep.tile([128, NDB * (D_CONV - 1)], F32)
    nc.vector.memset(tails[:], 0.0)

    x_dram = ios["x"]
    out_dram = ios["out"]
    win_dram = ios["w_inT"]
    wout_dram = ios["w_outT"]
    dt_scr = ios["dt_scr"]
    xm_scr = ios["xm_scr"]
    z_scr = ios["z_scr"]
    bc_scr = ios["bc_scr"]

    ts0 = TC // 128  # t-subtiles per chunk

    def emit_A1(tci):
        t0 = tci * TC
        # ===== A1: layernorm + transpose -> hT (bf16, [d] x [t-chunk]) =====
        ht = [htp.tile([128, TC], BF16, tag="ht", name=f"ht{tci}_{i}")
              for i in range(NKD)]
        for tt in range(ts0):
            xt = xpool.tile([128, D_MODEL], F32)
            nc.sync.dma_start(xt[:], x_dram[t0 + tt * 128: t0 + (tt + 1) * 128, :])
            red = lnsml.tile([128, 1], F32, tag="red")
            cen = cpool.tile([128, D_MODEL], F32)
            nc.scalar.activation(cen[:], xt[:], AF.Copy, accum_out=red[:])
            mu = lnsml.tile([128, 1], F32, tag="mu")
            nc.scalar.mul(mu[:], red[:], 1.0 / D_MODEL)
            nc.vector.tensor_scalar_sub(cen[:], cen[:], mu[:])
            ssq = lnsml.tile([128, 1], F32, tag="ssq")
            nc.scalar.activation(xt[:], cen[:], AF.Square, accum_out=ssq[:])
            lv = lnsml.tile([128, 1], F32, tag="lv")
            nc.scalar.activation(lv[:], ssq[:], AF.Ln, bias=epsb[:],
                                 scale=1.0 / D_MODEL)
            rstd = lnsml.tile([128, 1], F32, tag="rstd")
            nc.scalar.activation(rstd[:], lv[:], AF.Exp, scale=-0.5)
            hh = hpool.tile([128, D_MODEL], BF16)
            if ln_simple:
                # ln_w == 1, ln_b == 0 (host-verified): one scalar multiply
                nc.vector.tensor_scalar_mul(hh[:], cen[:], rstd[:])
            else:
                nc.vector.scalar_tensor_tensor(hh[:], cen[:], rstd[:], lnw[:],
                                               op0=ALU.mult, op1=ALU.mult)
                nc.vector.tensor_add(hh[:], hh[:], lnb[:])
            for kd in range(NKD):
                tp = tpsum.tile([128, 128], BF16, tag="tp", bufs=1)
                nc.tensor.transpose(tp[:], hh[:, kd * 128:(kd + 1) * 128], ident[:])
                nc.scalar.copy(ht[kd][:, tt * 128:(tt + 1) * 128], tp[:])

        return ht

    def emit_A2(tci, ht):
        t0 = tci * TC
        # ===== A2/A3: in_proj + conv + silu; A4: x_proj/dt_proj =====
        # eg 0-1 produce xm (and x_proj); the dt chain is emitted before the
        # z path (eg 2-3) so phase B's inputs land in DRAM as early as possible.
        xpps = xpp.tile([96, TC], F32)
        for eg in [0, 1, "dt", 2, 3]:
            if eg == "dt":
                bc32 = btct.tile([32, TC], BF16, tag="bc32")
                nc.scalar.copy(bc32[:], xpps[64:96, :])
                nc.sync.dma_start(bc_scr[tci][:], bc32[:])
                dtr = dtrp.tile([64, TC], BF16)
                nc.vector.tensor_copy(dtr[:], xpps[0:64, :])
                for dblk in range(NDB):
                    dps = xzp.tile([128, TC], F32, tag="ps", name="dps")
                    nc.tensor.matmul(dps[:], wdt[:, dblk * 128:(dblk + 1) * 128],
                                     dtr[:], start=True, stop=True)
                    dtt = dttp.tile([128, TC], F32)
                    nc.scalar.activation(dtt[:], dps[:], AF.Exp,
                                         bias=dtb[:, dblk:dblk + 1])
                    nc.scalar.activation(dtt[:], dtt[:], AF.Ln, bias=1.0)
                    nc.sync.dma_start(dt_scr[tci][dblk][:], dtt[:])
                continue
            wts = [winp.tile([128, 1024], BF16, tag="win",
                             name=f"win{tci}_{eg}_{i}") for i in range(NKD)]
            for kd in range(NKD):
                nc.sync.dma_start(
                    wts[kd][:],
                    win_dram[kd * 128:(kd + 1) * 128, eg * 1024:(eg + 1) * 1024])
            for me in range(8):
                e_idx = eg * 8 + me
                ps = xzp.tile([128, TC], F32)
                for kd in range(NKD):
                    nc.tensor.matmul(ps[:], wts[kd][:, me * 128:(me + 1) * 128],
                                     ht[kd][:], start=(kd == 0), stop=(kd == NKD - 1))
                if e_idx < NDB:
                    dblk = e_idx
                    # causal conv over time with halo
                    raw = rawp.tile([128, TC + 3], F32)
                    nc.scalar.copy(raw[:, 0:3], tails[:, dblk * 3:dblk * 3 + 3])
                    nc.scalar.copy(raw[:, 3:TC + 3], ps[:])
                    nc.scalar.copy(tails[:, dblk * 3:dblk * 3 + 3],
                                   raw[:, TC:TC + 3])
                    acc = accp.tile([128, TC], F32, tag="cacc")
                    nc.vector.tensor_scalar(acc[:], raw[:, 0:TC],
                                            conv4[:, dblk * 4:dblk * 4 + 1],
                                            convb[:, dblk:dblk + 1],
                                            op0=ALU.mult, op1=ALU.add)
                    for k in range(1, D_CONV):
                        acc2 = accp.tile([128, TC], F32 if k < D_CONV - 1
                                         else BF16, tag="cacc")
                        nc.vector.scalar_tensor_tensor(
                            acc2[:], raw[:, k:TC + k],
                            conv4[:, dblk * 4 + k:dblk * 4 + k + 1], acc[:],
                            op0=ALU.mult, op1=ALU.add)
                        acc = acc2
                    e1 = accp.tile([128, TC], F32, tag="sil1", name="sil1")
                    nc.scalar.activation(e1[:], acc[:], AF.Exp, scale=-1.0)
                    nc.scalar.activation(e1[:], e1[:], AF.Ln, bias=1.0)
                    sigb = accp.tile([128, TC], BF16, tag="sigb", name="sigb")
                    nc.scalar.activation(sigb[:], e1[:], AF.Exp, scale=-1.0)
                    xm = xmp.tile([128, TC], BF16)
                    nc.vector.tensor_mul(xm[:], acc[:], sigb[:])
                    nc.sync.dma_start(xm_scr[tci][dblk][:], xm[:])
                    nc.tensor.matmul(xpps[:], wx[:, dblk * 96:(dblk + 1) * 96],
                                     xm[:], start=(dblk == 0), stop=(dblk == NDB - 1))
                else:
                    dblk = e_idx - NDB
                    sig = _silu3(nc, accp, ps[:], TC)
                    zs = zp.tile([128, TC], BF16)
                    nc.vector.tensor_mul(zs[:], ps[:], sig[:])
                    nc.sync.dma_start(z_scr[tci][dblk][:], zs[:])

    def emit_B_bcast(tci):
        # ===== B: selective scan - B/C broadcasts =====
        br = []
        for n in range(D_STATE):
            bt = brep.tile([128, TC], BF16, tag="brep")
            nc.sync.dma_start(
                bt[:], bc_scr[tci][n:n + 1, :].to_broadcast((128, TC)))
            br.append(bt)
        csup = crep.tile([128, D_STATE * TC], BF16, tag="csup")
        for n in range(D_STATE):
            nc.sync.dma_start(
                csup[:, n * TC:(n + 1) * TC],
                bc_scr[tci][16 + n:17 + n, :].to_broadcast((128, TC)))
        return br, csup

    def emit_B_half(tci, br, csup, yg_list, dblks):
        for dblk in dblks:
            dtl = ldp.tile([128, TC], F32, tag="dtl")
            nc.sync.dma_start(dtl[:], dt_scr[tci][dblk][:])
            xml = ldp.tile([128, TC], BF16, tag="xml")
            nc.sync.dma_start(xml[:], xm_scr[tci][dblk][:])
            zl = ldp.tile([128, TC], BF16, tag="zl")
            nc.sync.dma_start(zl[:], z_scr[tci][dblk][:])
            dtb16 = up.tile([128, TC], BF16, tag="dtb16")
            nc.scalar.copy(dtb16[:], dtl[:])
            ut = up.tile([128, TC], BF16)
            nc.vector.tensor_mul(ut[:], dtb16[:], xml[:])
            hd = hdp.tile([128, D_STATE * TC], BF16)
            for n in range(D_STATE):
                da = dap.tile([128, TC], BF16)
                nc.scalar.activation(da[:], dtl[:], AF.Exp, scale=a_scales[n])
                dbx = dbxp.tile([128, TC], BF16)
                nc.vector.tensor_mul(dbx[:], ut[:], br[n][:])
                nc.vector.tensor_tensor_scan(
                    hd[:, n * TC:(n + 1) * TC], da[:], dbx[:],
                    initial=S[:, dblk * D_STATE + n:dblk * D_STATE + n + 1],
                    op0=ALU.mult, op1=ALU.add)
            # save last column of each scan as the next chunk's initial state
            hview = hd[:].rearrange("p (n t) -> p n t", n=D_STATE)
            nc.scalar.copy(S[:, dblk * D_STATE:(dblk + 1) * D_STATE],
                           hview[:, :, TC - 1])
            # C-contraction: one fused multiply + log-tree adds over n
            w = yaccp.tile([128, D_STATE * TC], BF16, tag="w")
            nc.vector.tensor_mul(w[:], hd[:], csup[:])
            half = D_STATE * TC // 2
            while half >= TC:
                nc.vector.tensor_add(w[:, 0:half], w[:, 0:half],
                                     w[:, half:2 * half])
                half //= 2
            ygs = ygsp.tile([128, TC], BF16)
            nc.vector.scalar_tensor_tensor(ygs[:], xml[:], dmat[:, dblk:dblk + 1],
                                           w[:, 0:TC], op0=ALU.mult, op1=ALU.add)
            yg = ygp.tile([128, TC], BF16, tag="yg")
            nc.vector.tensor_mul(yg[:], ygs[:], zl[:])
            yg_list.append(yg)

    def emit_C(tci, yg_list):
        # ===== C: out_proj + residual =====
        t0 = tci * TC
        for dch in range(D_MODEL // 512):
            opsl = [outps.tile([128, 512], F32, tag="ops",
                               name=f"ops{tci}_{dch}_{i}") for i in range(ts0)]
            for dblk in range(NDB):
                wo = woutp.tile([128, 512], BF16, tag="wo")
                nc.sync.dma_start(
                    wo[:], wout_dram[dblk * 128:(dblk + 1) * 128,
                                     dch * 512:(dch + 1) * 512])
                for tt in range(ts0):
                    nc.tensor.matmul(
                        opsl[tt][:], yg_list[dblk][:, tt * 128:(tt + 1) * 128],
                        wo[:], start=(dblk == 0), stop=(dblk == NDB - 1),
                        skip_group_check=True)
            for tt in range(ts0):
                xr = xresp.tile([128, 512], F32)
                nc.sync.dma_start(
                    xr[:], x_dram[t0 + tt * 128:t0 + (tt + 1) * 128,
                                  dch * 512:(dch + 1) * 512])
                ob = outsb.tile([128, 512], F32)
                nc.vector.tensor_add(ob[:], opsl[tt][:], xr[:])
                nc.sync.dma_start(
                    out_dram[t0 + tt * 128:t0 + (tt + 1) * 128,
                             dch * 512:(dch + 1) * 512], ob[:])

    # Software-pipelined emission: chunk k+1's phase-A pieces are emitted
    # interleaved INTO chunk k's scan phase, so each engine's (in-order)
    # instruction stream mixes next-chunk prep with the DVE-bound scan.
    ntc = L // TC
    ht0 = emit_A1(0)
    emit_A2(0, ht0)
    for tci in range(ntc):
        ygl = []
        br, csup = emit_B_bcast(tci)
        if tci + 1 < ntc:
            ht_next = emit_A1(tci + 1)
            emit_A2(tci + 1, ht_next)
        emit_B_half(tci, br, csup, ygl, range(NDB))
        emit_C(tci, ygl)


def _patch_act_tables():
    """Force every activation onto the single exp+ln table set.

    bacc's insert_act_table_loads assigns each activation the first table
    containing its func; with exp in set 0 and ln only in sets 5/6 the ACT
    engine would reload tables constantly (~1.3us each). Blanking every set
    except natural_log_exp_and_others (which holds exp/ln/square/copy/
    identity - all funcs this kernel uses) pins one table for the whole
    program while keeping act_func_set ids aligned with act_info.json.
    """
    import concourse.bacc as bacc_mod
    if getattr(bacc_mod, "_ant_single_table_patch", False):
        return
    orig = bacc_mod.get_activation_tables

    def patched(arch):
        tabs = orig(arch)
        keep = "natural_log_exp_and_others"
        out = {}
        for name, funcs in tabs.items():
            out[name] = funcs if name == keep else set()
        return out

    bacc_mod.get_activation_tables = patched
    bacc_mod._ant_single_table_patch = True


def build_program(a_scales, ln_simple=False):
    """Build + compile the single-core SPMD program. Returns nc."""
    _patch_act_tables()
    nc = bacc.Bacc("TRN2", target_bir_lowering=False, debug=False, num_devices=NB)
    ios = {}
    ios["x"] = nc.dram_tensor("x", [L, D_MODEL], F32, kind="ExternalInput")
    ios["w_inT"] = nc.dram_tensor("w_inT", [D_MODEL, 2 * D_INNER], BF16,
                                  kind="ExternalInput")
    ios["wxT"] = nc.dram_tensor("wxT", [128, NDB * 96], BF16, kind="ExternalInput")
    ios["wdtT"] = nc.dram_tensor("wdtT", [64, D_INNER], BF16, kind="ExternalInput")
    ios["w_outT"] = nc.dram_tensor("w_outT", [D_INNER, D_MODEL], BF16,
                                   kind="ExternalInput")
    ios["conv4"] = nc.dram_tensor("conv4", [128, NDB * D_CONV], F32,
                                  kind="ExternalInput")
    ios["convb"] = nc.dram_tensor("convb", [128, NDB], F32, kind="ExternalInput")
    ios["dtb"] = nc.dram_tensor("dtb", [128, NDB], F32, kind="ExternalInput")
    ios["dmat"] = nc.dram_tensor("dmat", [128, NDB], F32, kind="ExternalInput")
    ios["lnw_rep"] = nc.dram_tensor("lnw_rep", [128, D_MODEL], BF16,
                                    kind="ExternalInput")
    ios["lnb_rep"] = nc.dram_tensor("lnb_rep", [128, D_MODEL], BF16,
                                    kind="ExternalInput")
    ios["ident"] = nc.dram_tensor("ident", [128, 128], BF16, kind="ExternalInput")
    ios["out"] = nc.dram_tensor("out", [L, D_MODEL], F32, kind="ExternalOutput")
    ntc = L // TC
    ios["dt_scr"] = [[nc.dram_tensor(f"dt_scr{t}_{d}", [128, TC], F32)
                      for d in range(NDB)] for t in range(ntc)]
    ios["xm_scr"] = [[nc.dram_tensor(f"xm_scr{t}_{d}", [128, TC], BF16)
                      for d in range(NDB)] for t in range(ntc)]
    ios["z_scr"] = [[nc.dram_tensor(f"z_scr{t}_{d}", [128, TC], BF16)
                     for d in range(NDB)] for t in range(ntc)]
    ios["bc_scr"] = [nc.dram_tensor(f"bc_scr{t}", [32, TC], BF16)
                     for t in range(ntc)]

    from contextlib import ExitStack
    with tile.TileContext(nc) as tc_ctx:
        with ExitStack() as ctx:
            _emit(tc_ctx, nc, ios, a_scales, ctx, ln_simple=ln_simple)
    nc.compile()
    return nc


def host_prep(inputs):
    """Host-side input preprocessing -> per-core in_maps."""
    bf = ml_dtypes.bfloat16
    x = np.asarray(inputs["x"], np.float32)
    A = -np.exp(np.asarray(inputs["A_log"], np.float32))  # (di, n)
    assert np.allclose(A, A[0:1, :], atol=1e-5), "A must be d-independent"
    a_scales = [float(A[0, n]) for n in range(D_STATE)]
    ln_simple = bool(np.all(np.asarray(inputs["ln_w"], np.float32) == 1.0)
                     and np.all(np.asarray(inputs["ln_b"], np.float32) == 0.0))

    def dmaj(v, cols):  # (D_INNER, cols) -> (128, NDB*cols) d-block-major
        return np.ascontiguousarray(
            v.reshape(NDB, 128, cols).transpose(1, 0, 2).reshape(128, NDB * cols))

    w_inT = np.ascontiguousarray(
        np.asarray(inputs["in_proj_w"], np.float32).T).astype(bf)
    wxT = dmaj(np.ascontiguousarray(np.asarray(inputs["x_proj_w"], np.float32).T),
               96).astype(bf)
    wdtT = np.ascontiguousarray(
        np.asarray(inputs["dt_proj_w"], np.float32).T).astype(bf)
    w_outT = np.ascontiguousarray(
        np.asarray(inputs["out_proj_w"], np.float32).T).astype(bf)
    conv4 = dmaj(np.asarray(inputs["conv_w"], np.float32).reshape(D_INNER, D_CONV),
                 D_CONV)
    convb = dmaj(np.asarray(inputs["conv_b"], np.float32).reshape(D_INNER, 1), 1)
    dtb = dmaj(np.asarray(inputs["dt_proj_b"], np.float32).reshape(D_INNER, 1), 1)
    dmat = dmaj(np.asarray(inputs["D"], np.float32).reshape(D_INNER, 1), 1)
    lnw_rep = np.ascontiguousarray(
        np.broadcast_to(np.asarray(inputs["ln_w"], np.float32)[None, :],
                        (128, D_MODEL))).astype(bf)
    lnb_rep = np.ascontiguousarray(
        np.broadcast_to(np.asarray(inputs["ln_b"], np.float32)[None, :],
                        (128, D_MODEL))).astype(bf)
    ident = np.eye(128, dtype=np.float32).astype(bf)

    shared = dict(w_inT=w_inT, wxT=wxT, wdtT=wdtT, w_outT=w_outT, conv4=conv4,
                  convb=convb, dtb=dtb, dmat=dmat, lnw_rep=lnw_rep,
                  lnb_rep=lnb_rep, ident=ident)
    in_maps = []
    for i in range(NB):
        m = dict(shared)
        m["x"] = np.ascontiguousarray(x[i])
        in_maps.append(m)
    return a_scales, in_maps, ln_simple


_CACHE = {}


def kernel(**inputs):
    a_scales, in_maps, ln_simple = host_prep(inputs)
    key = (tuple(a_scales), ln_simple)
    if key not in _CACHE:
        _CACHE[key] = build_program(a_scales, ln_simple)
    nc = _CACHE[key]
    res = run_bass_kernel_spmd(nc, in_maps, list(range(NB)))
    out = np.stack([np.asarray(r["out"], np.float32) for r in res.results])
    return out

